# revision 17
# baseline (speedup 1.0000x reference)
"""Trainium2 Bass kernel for the DQN hypergraph-conv network (8-core SPMD).

Sharding: edges row-sharded for the message stage (Hs@X@theta local per
edge shard), nodes column-sharded for the aggregation stage (Ht.T @ ...),
with AllGather collectives moving the small [E,H]/[N,H] intermediates.
The big Ht/Hs shards are read once in bf16 and stay resident in SBUF
across both conv layers.

Per core c (NCORES=8):
  hsT = Hs[e_c, :].T   [N, E/8]  bf16   (stage-1 moving operand)
  ht  = Ht[:, n_c]     [E, N/8]  bf16   (stage-2 moving operand)
  stage1: tmpT[f,e] = sum_n X[n-tile].T @ hsT[n-tile]      (PE, N=512 free)
  msg[e,h] = tmpT.T @ theta ; scaled = edge_w * msg        -> AllGather
  stage2: aggT[h,n] = w_trans.T @ xiT + bias (rank-1)
          + sum_e scaled[e-tile].T @ ht[e-tile]            (PE, N=512 free)
  epilogue: fused leaky-relu / dropout mask / second lrelu  (DVE)
  conv0 only: PE-transpose X1T -> X1 tiles -> 2-chunk AllGather
  fc: fc_w.T @ XT (f32 matmul) + host-precomputed state term -> [1, N/8]

DMA layout notes: every dma_start costs ~0.6-2us of serial issue time on
the triggering engine, and a single InstDMACopy is already split across
all 16 SDMA engines, so transfers are consolidated into a few large DMAs
and spread across both HWDGE rings (sync + scalar). All small parameters
are packed host-side into one bf16 and one f32 tensor.
"""

import sys

if "/opt/trn_rl_repo" not in sys.path:
    sys.path.insert(0, "/opt/trn_rl_repo")

import numpy as np
import ml_dtypes

NCORES = 8
N, E, F = 8192, 4096, 128
E_SH = E // NCORES   # 512 edges per core
N_SH = N // NCORES   # 1024 nodes per core
NEG_SLOPE = 0.01
DROP_P = 0.5

# packed bf16 params layout (columns)
PB_TH = 0          # th0, th1         [128, 128] each
PB_WT = 256        # wt0, wt1
PB_IDN = 512       # identity
PB_XIT = 640       # xiT              [128, 1024]
PB_B = 1664        # b0, b1 on partition 0, 128 cols each
PB_W = 1920
# packed f32 params layout (columns)
PF_EW = 0          # ew0, ew1         [128, 4] each
PF_M2T = 8         # mask2T           [128, 1024]
PF_FCW = 1032      # fc_w[:128]       [128, 1]
PF_ST = 1033       # state term on partition 0, 1024 cols
PF_W = 2060

_CACHE = {}


def _build_nc():
    import concourse.bacc as bacc
    import concourse.mybir as mybir
    import concourse.tile as tile
    from concourse.tile import add_dep_helper

    bf16 = mybir.dt.bfloat16
    f32 = mybir.dt.float32
    Alu = mybir.AluOpType

    nc = bacc.Bacc("TRN2", target_bir_lowering=False, debug=False,
                   num_devices=NCORES)

    hsT_d = nc.dram_tensor("hsT", [N, E_SH], bf16, kind="ExternalInput")
    ht_d = nc.dram_tensor("ht", [E, N_SH], bf16, kind="ExternalInput")
    xbf_d = nc.dram_tensor("xbf", [N, F], bf16, kind="ExternalInput")
    pbf_d = nc.dram_tensor("pbf", [128, PB_W], bf16, kind="ExternalInput")
    pf32_d = nc.dram_tensor("pf32", [128, PF_W], f32, kind="ExternalInput")
    out_d = nc.dram_tensor("out", [1, N_SH], f32, kind="ExternalOutput")

    RG = [list(range(NCORES))]

    with tile.TileContext(nc) as tc:
        with (
            tc.tile_pool(name="sb", bufs=1) as sb,
            tc.tile_pool(name="sc2", bufs=2) as sc2,
            tc.tile_pool(name="ps_tmp", bufs=1, space="PSUM") as ps_tmp,
            tc.tile_pool(name="ps_agg", bufs=2, space="PSUM") as ps_agg,
            tc.tile_pool(name="ps_sm", bufs=2, space="PSUM") as ps_sm,
            tc.tile_pool(name="dram", bufs=1, space="DRAM") as dram,
        ):
            # ---- packed params (scalar ring) ----
            pbf = sb.tile([128, PB_W], bf16, tag="pbf")
            nc.scalar.dma_start(pbf[:], pbf_d[:, :])
            pf = sb.tile([128, PF_W], f32, tag="pf")
            nc.scalar.dma_start(pf[:], pf32_d[:, :])

            def th(conv):
                return pbf[:, PB_TH + conv * 128:PB_TH + (conv + 1) * 128]

            def wt(conv):
                return pbf[:, PB_WT + conv * 128:PB_WT + (conv + 1) * 128]

            idn = pbf[:, PB_IDN:PB_IDN + 128]
            xiT = pbf[:, PB_XIT:PB_XIT + 1024]

            def bias(conv):
                return pbf[0:1, PB_B + conv * 128:PB_B + (conv + 1) * 128]

            def ew(conv, ec):
                c0 = PF_EW + conv * 4 + ec
                return pf[:, c0:c0 + 1]

            m2T = pf[:, PF_M2T:PF_M2T + 1024]
            fcw = pf[:, PF_FCW:PF_FCW + 1]
            stT = pf[0:1, PF_ST:PF_ST + 1024]

            ones_sb = sb.tile([1, 512], bf16, tag="ones")
            nc.vector.memset(ones_sb[:], 1.0)

            # ---- big resident loads ----
            # DMA queues fair-share bandwidth, so unchained concurrent loads
            # all complete together at the end.  Chain them (each waits on
            # the previous) so chunks arrive pipelined, in the order stage 1
            # consumes them, each at full single-DMA bandwidth.
            x_r = xbf_d.ap().rearrange("(b p) h -> p b h", p=128)
            hsT_r = hsT_d.ap().rearrange("(b p) e -> p b e", p=128)
            xall = [sb.tile([128, 32, F], bf16, tag=f"x{i}", name=f"x{i}")
                    for i in range(2)]
            hsT_t = [sb.tile([128, 16, E_SH], bf16, tag=f"hsT{i}",
                             name=f"hsT{i}") for i in range(4)]
            chain = [
                (xall[0], x_r[:, 0:32, :]),
                (hsT_t[0], hsT_r[:, 0:16, :]),
                (hsT_t[1], hsT_r[:, 16:32, :]),
                (xall[1], x_r[:, 32:64, :]),
                (hsT_t[2], hsT_r[:, 32:48, :]),
                (hsT_t[3], hsT_r[:, 48:64, :]),
            ]
            prev = None
            for dst, src in chain:
                dma = nc.sync.dma_start(dst[:], src)
                if prev is not None:
                    add_dep_helper(dma.ins, prev.ins, sync=True,
                                   reason="pipeline load chain")
                prev = dma
            last_load = prev

            ht_t = [None, None]
            ht_r = ht_d.ap().rearrange("(b p) n -> p b n", p=128)

            # collective bounce buffers (msg and X1 AllGathers split in two
            # chunks each so downstream compute starts on chunk 0 while
            # chunk 1 is still on the wire)
            agm_in = [dram.tile([E_SH, F], bf16, tag=f"agmi{i}",
                                name=f"agmi{i}") for i in range(2)]
            agm_out = [[dram.tile([E // 2, F], bf16, addr_space="Shared",
                                  tag=f"agmo{i}{j}", name=f"agmo{i}{j}")
                        for j in range(2)] for i in range(2)]
            agx_in = dram.tile([N_SH, F], bf16, tag="agxi")
            agx_out = [dram.tile([N // 2, F], bf16, addr_space="Shared",
                                 tag=f"agxo{j}", name=f"agxo{j}")
                       for j in range(2)]

            x1c = [None, None]   # gathered X1 chunks for conv1 stage 1
            x1t_tiles = [None, None]
            xT = [None, None]    # final-layer activations (f32)

            for conv in range(2):
                # ---------- stage 1: tmpT = X.T @ HsT ----------
                tmpT_ps = ps_tmp.tile([128, E_SH], f32, tag="tmpT")
                mm_last = None
                if conv == 0:
                    for nt in range(64):
                        mm_last = nc.tensor.matmul(
                            tmpT_ps[:], xall[nt // 32][:, nt % 32, :],
                            hsT_t[nt // 16][:, nt % 16, :],
                            start=(nt == 0), stop=(nt == 63))
                else:
                    # consume gathered X1 chunk 0 first, then chunk 1
                    cnt = 0
                    for j in range(2):
                        for b in range(32):
                            nt = (b // 4) * 8 + j * 4 + (b % 4)
                            mm_last = nc.tensor.matmul(
                                tmpT_ps[:], x1c[j][:, b, :],
                                hsT_t[nt // 16][:, nt % 16, :],
                                start=(cnt == 0), stop=(cnt == 63))
                            cnt += 1

                if conv == 0:
                    # ht loads deferred behind stage 1 so they don't steal
                    # HBM bandwidth from the critical hsT/x stream; they
                    # fill the AllGather window instead.
                    for i in range(2):
                        hc = sb.tile([128, 16, N_SH], bf16, tag=f"ht{i}")
                        dma = nc.scalar.dma_start(
                            hc[:], ht_r[:, i * 16:(i + 1) * 16, :])
                        add_dep_helper(dma.ins, mm_last.ins, sync=True,
                                       reason="defer ht behind stage1")
                        ht_t[i] = hc

                tmpT_bf = sb.tile([128, E_SH], bf16, tag=f"tmpTbf{conv}")
                nc.vector.tensor_copy(tmpT_bf[:], tmpT_ps[:])

                # ---------- msg = tmpT.T @ theta, scaled by edge_w ----------
                # 2-chunk AllGather: ship edges [0:256] while [256:512]
                # are still being computed.
                msg_sb = sb.tile([128, 4, F], bf16, tag="msg")
                agm_r = agm_in[conv].rearrange("(c p) h -> p c h", p=128)
                for ec in range(4):
                    mps = ps_sm.tile([128, F], f32, tag="msg", bufs=2)
                    nc.tensor.matmul(
                        mps[:], tmpT_bf[:, ec * 128:(ec + 1) * 128],
                        th(conv), start=True, stop=True)
                    nc.vector.tensor_scalar(
                        msg_sb[:, ec, :], mps[:], ew(conv, ec), None, Alu.mult)
                    if ec % 2 == 1:
                        j = ec // 2
                        nc.sync.dma_start(agm_r[:, j * 2:(j + 1) * 2, :],
                                          msg_sb[:, j * 2:(j + 1) * 2, :])
                        nc.gpsimd.collective_compute(
                            "AllGather", Alu.bypass, replica_groups=RG,
                            ins=[agm_in[conv][j * 256:(j + 1) * 256, :]],
                            outs=[agm_out[conv][j][:]])

                # gathered scaled msg chunks: chunk j block b -> rank b//2,
                # within-chunk tile b%2 -> global edge tile (b//2)*4 + j*2 + b%2
                sc_t = []
                for j in range(2):
                    sc = sb.tile([128, 16, F], bf16, tag=f"sc{j}",
                                 name=f"sc{j}")
                    nc.scalar.dma_start(
                        sc[:], agm_out[conv][j].rearrange(
                            "(b p) h -> p b h", p=128))
                    sc_t.append(sc)

                # ---------- stage 2: aggT = wT@xiT + b + scaled.T @ Ht ----------
                for nb in range(2):
                    agg = ps_agg.tile([128, 512], f32, tag="agg")
                    nc.tensor.matmul(
                        agg[:], wt(conv), xiT[:, nb * 512:(nb + 1) * 512],
                        start=True, stop=False)
                    nc.tensor.matmul(
                        agg[:], bias(conv), ones_sb[:],
                        start=False, stop=False)
                    cnt = 0
                    for j in range(2):
                        for b in range(16):
                            et = (b // 2) * 4 + j * 2 + (b % 2)
                            cnt += 1
                            nc.tensor.matmul(
                                agg[:], sc_t[j][:, b, :],
                                ht_t[et // 16][:, et % 16,
                                               nb * 512:(nb + 1) * 512],
                                start=False, stop=(cnt == 32))

                    if conv == 0:
                        # X1T = lrelu(agg) * dropout_mask  -> transpose ->
                        # bounce -> AllGather chunk nb (pipelined per nb)
                        sl = sc2.tile([128, 512], f32, tag="sl")
                        nc.vector.tensor_scalar(
                            sl[:], agg[:], NEG_SLOPE, None, Alu.mult)
                        lr = sc2.tile([128, 512], f32, tag="lr")
                        nc.vector.tensor_tensor(lr[:], agg[:], sl[:], Alu.max)
                        x1t = sb.tile([128, 512], bf16, tag=f"x1t{nb}")
                        nc.vector.tensor_tensor(
                            x1t[:], lr[:], m2T[:, nb * 512:(nb + 1) * 512],
                            Alu.mult)

                        x1loc = sb.tile([128, 4, F], bf16, tag=f"x1loc{nb}",
                                        name=f"x1loc{nb}")
                        for t in range(4):
                            tps = ps_sm.tile([128, 128], bf16, tag="tr",
                                             bufs=2)
                            nc.tensor.transpose(
                                tps[:], x1t[:, t * 128:(t + 1) * 128], idn)
                            nc.vector.tensor_copy(x1loc[:, t, :], tps[:])
                        agx_r = agx_in.rearrange("(c p) h -> p c h", p=128)
                        nc.sync.dma_start(
                            agx_r[:, nb * 4:(nb + 1) * 4, :], x1loc[:])
                        nc.gpsimd.collective_compute(
                            "AllGather", Alu.bypass, replica_groups=RG,
                            ins=[agx_in[nb * 512:(nb + 1) * 512, :]],
                            outs=[agx_out[nb][:]])
                        xc = sb.tile([128, 32, F], bf16, tag=f"x1c{nb}",
                                     name=f"x1c{nb}")
                        nc.scalar.dma_start(
                            xc[:], agx_out[nb].rearrange(
                                "(b p) h -> p b h", p=128))
                        x1c[nb] = xc
                    else:
                        # X = lrelu(lrelu(agg)) = max(agg, 1e-4*agg)  (f32)
                        sl = sc2.tile([128, 512], f32, tag="sl")
                        nc.vector.tensor_scalar(
                            sl[:], agg[:], NEG_SLOPE * NEG_SLOPE, None,
                            Alu.mult)
                        t = sb.tile([128, 512], f32, tag=f"xT{nb}")
                        nc.vector.tensor_tensor(t[:], agg[:], sl[:], Alu.max)
                        xT[nb] = t
                        # fc for this block immediately
                        fps = ps_sm.tile([1, 512], f32, tag="fc", bufs=1)
                        nc.tensor.matmul(fps[:], fcw, t[:],
                                         start=True, stop=True)
                        osb = sc2.tile([1, 512], f32, tag="osb")
                        nc.vector.tensor_tensor(
                            osb[:], fps[:], stT[:, nb * 512:(nb + 1) * 512],
                            Alu.add)
                        nc.sync.dma_start(
                            out_d[0:1, nb * 512:(nb + 1) * 512], osb[:])

    nc.compile()
    return nc


def _get_nc():
    if "nc" not in _CACHE:
        _CACHE["nc"] = _build_nc()
    return _CACHE["nc"]


def _dropout_mask2():
    """2.0 * bernoulli(key(42), 0.5, (N, F)) exactly as the reference."""
    import jax
    cpu = jax.devices("cpu")[0]
    with jax.default_device(cpu):
        keep = jax.random.bernoulli(jax.random.key(42), 1.0 - DROP_P, (N, F))
        return np.asarray(keep).astype(np.float32) * (1.0 / (1.0 - DROP_P))


def prepare_in_maps(xi, x, Ht, Hs, state,
                    w_trans0, theta0, edge_w0, bias0,
                    w_trans1, theta1, edge_w1, bias1,
                    fc_w, fc_b):
    bf = ml_dtypes.bfloat16
    mask2 = _dropout_mask2()

    xbf = np.ascontiguousarray(x, np.float32).astype(bf)
    fcw32 = np.asarray(fc_w, np.float32)
    fcw_last = float(fcw32[F, 0])
    fcb = float(np.asarray(fc_b, np.float32)[0])

    Hs32 = np.asarray(Hs, np.float32)
    Ht32 = np.asarray(Ht, np.float32)
    xi32 = np.asarray(xi, np.float32)
    st32 = np.asarray(state, np.float32)
    th = [np.asarray(theta0, np.float32), np.asarray(theta1, np.float32)]
    wtr = [np.asarray(w_trans0, np.float32), np.asarray(w_trans1, np.float32)]
    bs = [np.asarray(bias0, np.float32), np.asarray(bias1, np.float32)]
    ews = [np.asarray(edge_w0, np.float32), np.asarray(edge_w1, np.float32)]

    in_maps = []
    for c in range(NCORES):
        e0, e1 = c * E_SH, (c + 1) * E_SH
        n0, n1 = c * N_SH, (c + 1) * N_SH

        pbf = np.zeros((128, PB_W), np.float32)
        pbf[:, PB_TH:PB_TH + 128] = th[0]
        pbf[:, PB_TH + 128:PB_TH + 256] = th[1]
        pbf[:, PB_WT:PB_WT + 128] = wtr[0]
        pbf[:, PB_WT + 128:PB_WT + 256] = wtr[1]
        pbf[:, PB_IDN:PB_IDN + 128] = np.eye(F)
        pbf[:, PB_XIT:PB_XIT + 1024] = xi32[n0:n1, :].T
        pbf[0, PB_B:PB_B + 128] = bs[0]
        pbf[0, PB_B + 128:PB_B + 256] = bs[1]

        pf = np.zeros((128, PF_W), np.float32)
        pf[:, PF_EW:PF_EW + 4] = ews[0][e0:e1].reshape(4, 128).T
        pf[:, PF_EW + 4:PF_EW + 8] = ews[1][e0:e1].reshape(4, 128).T
        pf[:, PF_M2T:PF_M2T + 1024] = mask2[n0:n1, :].T
        pf[:, PF_FCW:PF_FCW + 1] = fcw32[:F, :]
        pf[0, PF_ST:PF_ST + 1024] = st32[n0:n1, 0] * fcw_last + fcb

        in_maps.append({
            "hsT": np.ascontiguousarray(Hs32[e0:e1, :].T).astype(bf),
            "ht": np.ascontiguousarray(Ht32[:, n0:n1]).astype(bf),
            "xbf": xbf,
            "pbf": pbf.astype(bf),
            "pf32": pf,
        })
    return in_maps


def kernel(xi, x, Ht, Hs, state,
           w_trans0, theta0, edge_w0, bias0,
           w_trans1, theta1, edge_w1, bias1,
           fc_w, fc_b, _trace=False):
    from concourse.bass_utils import run_bass_kernel_spmd

    nc = _get_nc()
    in_maps = prepare_in_maps(
        xi, x, Ht, Hs, state,
        w_trans0, theta0, edge_w0, bias0,
        w_trans1, theta1, edge_w1, bias1,
        fc_w, fc_b)
    res = run_bass_kernel_spmd(
        nc, in_maps, core_ids=list(range(NCORES)), trace=_trace)
    if _trace:
        _CACHE["last_results"] = res
    out = np.concatenate(
        [res.results[c]["out"].reshape(N_SH) for c in range(NCORES)])
    return out.reshape(N, 1).astype(np.float32)
